# revision 37
# baseline (speedup 1.0000x reference)
"""Multi-head causal attention (QKV proj + attention + O proj) on 8 TRN2 cores.

Sharding: data-parallel over batch (4) x tensor-parallel over heads (2 groups
of 8 heads).  Core c handles batch c//2, head-group c%2.  Each core computes
its group's partial o_proj output; the host sums the two partials per batch.

Layout strategy (all activations arrive pre-transposed from the host):
  - qT, kT per head-pair M-tile: (128 head-dims, L) from  W.T-slice @ X.T
    (1/sqrt(d_k) is folded into Wq/bq on the host).
  - vA natural (tokens, head-dims) with 64 REPLICATED ones-columns per head:
    av_psum = vA.T @ P gives (128, Nq) where rows 64:128 all hold the per-query
    sum of probabilities.  Normalization = vector reciprocal of rows 64:128 +
    one elementwise mult -- no DRAM round trips, no cross-partition broadcast.
  - AV accumulates over ALL key blocks in a single PSUM bank (no SBUF
    eviction), exp without max-subtraction (scores are O(+-6), safe in fp32),
    causal masking multiplies a small triangular mask after exp.
  - Attention inner loop is software-pipelined: scores(kb+1) and interleaved
    projection matmuls are emitted between scores(kb) and AV(kb) so the
    in-order tensor queue never stalls behind the scalar-engine exp.
Compute dtype bf16 (fp32 PSUM accumulation); fp32 partial outputs.
"""

import numpy as np
import ml_dtypes

import concourse.bass as bass
import concourse.tile as tile
from concourse import bacc, mybir

D_MODEL = 1024
N_HEADS = 16
D_K = 64
B, L = 4, 2048
TP = 2                  # head groups
GD = D_MODEL // TP      # 512 head-dims per group
P = 128
NQ = 512                # query chunk (one fp32 PSUM bank)
N_MT = GD // P          # 4 M-tiles (head pairs) per group
N_KT = D_MODEL // P     # 8 contraction tiles over model dim
N_TT = L // P           # 16 token tiles
N_QC = L // NQ          # 4 query chunks
BF16 = mybir.dt.bfloat16
F32 = mybir.dt.float32
NPBF16 = ml_dtypes.bfloat16
AF = mybir.ActivationFunctionType
ALU = mybir.AluOpType


def build_nc() -> bass.Bass:
    nc = bacc.Bacc("TRN2", target_bir_lowering=False)

    xqT = nc.dram_tensor("xqT", [D_MODEL, L], BF16, kind="ExternalInput")
    xkT = nc.dram_tensor("xkT", [D_MODEL, L], BF16, kind="ExternalInput")
    xvT = nc.dram_tensor("xvT", [D_MODEL, L], BF16, kind="ExternalInput")
    wqT = nc.dram_tensor("wqT", [D_MODEL, GD], BF16, kind="ExternalInput")
    wkT = nc.dram_tensor("wkT", [D_MODEL, GD], BF16, kind="ExternalInput")
    wvT = nc.dram_tensor("wvT", [D_MODEL, GD], BF16, kind="ExternalInput")
    woT = nc.dram_tensor("woT", [GD, D_MODEL], BF16, kind="ExternalInput")
    bq = nc.dram_tensor("bq", [P, N_MT], F32, kind="ExternalInput")
    bk = nc.dram_tensor("bk", [P, N_MT], F32, kind="ExternalInput")
    bv = nc.dram_tensor("bv", [1, GD], F32, kind="ExternalInput")
    maskc = nc.dram_tensor("maskc", [P, P], BF16, kind="ExternalInput")
    out = nc.dram_tensor("out", [L, D_MODEL], F32, kind="ExternalOutput")

    with tile.TileContext(nc) as tc:
        with (
            tc.tile_pool(name="const", bufs=1) as const,
            tc.tile_pool(name="xch", bufs=20) as xch_pool,
            tc.tile_pool(name="xvc", bufs=12) as xvc_pool,
            tc.tile_pool(name="pt", bufs=6) as pt_pool,
            tc.tile_pool(name="rec", bufs=4) as rec_pool,
            tc.tile_pool(name="osb", bufs=4) as osb_pool,
            tc.tile_pool(name="ps_s", bufs=2, space="PSUM") as ps_s,
            tc.tile_pool(name="ps_av", bufs=2, space="PSUM") as ps_av,
            tc.tile_pool(name="ps_mm", bufs=2, space="PSUM") as ps_mm,
        ):
            # ---- resident weights / constants ----
            wk_sb = const.tile([P, N_KT, GD], BF16, tag="wk")
            wv_sb = const.tile([P, N_KT, GD], BF16, tag="wv")
            wq_sb = const.tile([P, N_KT, GD], BF16, tag="wq")
            wo_sb = const.tile([P, N_MT, D_MODEL], BF16, tag="wo")
            bk_sb = const.tile([P, N_MT], F32, tag="bk")
            bq_sb = const.tile([P, N_MT], F32, tag="bq")
            bv_sb = const.tile([P, GD], F32, tag="bv")
            mask_sb = const.tile([P, P], BF16, tag="mask")

            # per-(mt, chunk) activation tiles
            qTt = [[const.tile([P, NQ], BF16, tag=f"qT{mt}_{ncz}", name=f"qT{mt}_{ncz}")
                    for ncz in range(N_QC)] for mt in range(N_MT)]
            kTt = [[const.tile([P, NQ], BF16, tag=f"kT{mt}_{ncz}", name=f"kT{mt}_{ncz}")
                    for ncz in range(N_QC)] for mt in range(N_MT)]
            # v with 64 replicated ones-columns per head: [tok, tt, head, 128]
            vA = const.tile([P, N_TT, 2 * N_MT, 2 * D_K], BF16, tag="vA")
            aoTq = [[const.tile([P, NQ], BF16, tag=f"ao{mt}_{qc}", name=f"ao{mt}_{qc}")
                     for qc in range(N_QC)] for mt in range(N_MT)]

            # ---- DMA issue: interleave weight/activation chunks so the first
            # projection chains start after ~256KB, not ~2MB; keep the scalar
            # queue free for exp ----
            nc.sync.dma_start(out=mask_sb, in_=maskc[:, :])
            nc.sync.dma_start(out=bk_sb, in_=bk[:, :])
            nc.sync.dma_start(out=bq_sb, in_=bq[:, :])
            nc.sync.dma_start(out=bv_sb, in_=bv[:, :].to_broadcast([P, GD]))

            xkc = {}   # (qc, kt) -> tile
            xqc = {}
            xvc = {}

            def x_chunk_dmas(dst, x_dram, nm, qc, eng):
                for kt in range(N_KT):
                    xc = (xvc_pool if nm == "v" else xch_pool).tile(
                        [P, NQ], BF16, tag="xvc" if nm == "v" else "xch",
                        name=f"x{nm}{qc}_{kt}")
                    eng.dma_start(
                        out=xc,
                        in_=x_dram[kt * P:(kt + 1) * P, qc * NQ:(qc + 1) * NQ],
                    )
                    dst[(qc, kt)] = xc

            # wk on gpsimd concurrently with xk0 on sync; then v, q inputs
            x_chunk_dmas(xkc, xkT, "k", 0, nc.sync)
            for kt in range(N_KT):
                nc.gpsimd.dma_start(out=wk_sb[:, kt, :],
                                    in_=wkT[kt * P:(kt + 1) * P, :])
            for kt in range(N_KT):
                nc.sync.dma_start(out=wv_sb[:, kt, :],
                                  in_=wvT[kt * P:(kt + 1) * P, :])
            x_chunk_dmas(xvc, xvT, "v", 0, nc.sync)
            x_chunk_dmas(xqc, xqT, "q", 0, nc.gpsimd)
            for kt in range(N_KT):
                nc.sync.dma_start(out=wq_sb[:, kt, :],
                                  in_=wqT[kt * P:(kt + 1) * P, :])
            for kt in range(N_MT):
                nc.sync.dma_start(out=wo_sb[:, kt, :],
                                  in_=woT[kt * P:(kt + 1) * P, :])

            # ones block of vA (cols 0:64 -> av rows 0:64 = replicated denom)
            nc.gpsimd.memset(vA[:, 0:N_TT // 4, :, 0:D_K], 1.0)
            nc.vector.memset(vA[:, N_TT // 4:N_TT, :, 0:D_K], 1.0)

            # ---- projection helpers ----
            def kq_chain(w_sb, b_sb, dsts, src, nm, qc, mt):
                """Returns list of emission closures: 8 matmuls + 1 evict."""
                ops = []
                ps = ps_mm.tile([P, NQ], F32, tag="mm", name=f"ps{nm}{qc}{mt}")

                def mm(kt):
                    def go():
                        nc.tensor.matmul(
                            ps,
                            lhsT=w_sb[:, kt, mt * P:(mt + 1) * P],
                            rhs=src[(qc, kt)],
                            start=(kt == 0),
                            stop=(kt == N_KT - 1),
                        )
                    return go
                for kt in range(N_KT):
                    ops.append(mm(kt))

                def evict():
                    nc.vector.tensor_scalar_add(
                        dsts[mt][qc], ps, b_sb[:, mt:mt + 1])
                ops.append(evict)
                return ops

            def v_chain(tt):
                """8 matmuls + 1 biased evict into vA[:, tt, :, 0:D_K]."""
                ops = []
                qc, j = tt // 4, tt % 4
                ps = ps_mm.tile([P, GD], F32, tag="mm", name=f"psv{tt}")

                def mm(kt):
                    def go():
                        nc.tensor.matmul(
                            ps,
                            lhsT=xvc[(qc, kt)][:, j * P:(j + 1) * P],
                            rhs=wv_sb[:, kt, :],
                            start=(kt == 0),
                            stop=(kt == N_KT - 1),
                        )
                    return go
                for kt in range(N_KT):
                    ops.append(mm(kt))

                def evict():
                    nc.vector.tensor_tensor(
                        out=vA[:, tt, :, D_K:2 * D_K],
                        in0=ps.rearrange("p (h d) -> p h d", d=D_K),
                        in1=bv_sb.rearrange("p (h d) -> p h d", d=D_K),
                        op=ALU.add,
                    )
                ops.append(evict)
                return ops

            def o_chain(qc, j, dc):
                """4 matmuls + 1 evict+store for out rows (4qc+j)*P, cols dc*NQ."""
                ops = []
                lt = 4 * qc + j
                ps = ps_mm.tile([P, NQ], F32, tag="mm", name=f"po{lt}_{dc}")

                def mm(kt):
                    def go():
                        nc.tensor.matmul(
                            ps,
                            lhsT=aoTq[kt][qc][:, j * P:(j + 1) * P],
                            rhs=wo_sb[:, kt, dc * NQ:(dc + 1) * NQ],
                            start=(kt == 0),
                            stop=(kt == N_MT - 1),
                        )
                    return go
                for kt in range(N_MT):
                    ops.append(mm(kt))

                def evict():
                    ot = osb_pool.tile([P, NQ], F32, tag="ot", name=f"ot{lt}_{dc}")
                    nc.vector.tensor_copy(out=ot, in_=ps)
                    nc.sync.dma_start(
                        out=out[lt * P:(lt + 1) * P, dc * NQ:(dc + 1) * NQ],
                        in_=ot,
                    )
                ops.append(evict)
                return ops

            # ---- attention with software-pipelined emission; `fillers` is a
            # shared list of closures drained evenly across the wave ----
            def av_batch(av, mt, batch, nkb):
                for h2 in range(2):
                    for (p3b, lob, kbb) in batch:
                        nc.tensor.matmul(
                            av[h2][:, lob:NQ],
                            lhsT=vA[:, kbb, 2 * mt + h2, :],
                            rhs=p3b[:, h2, lob:NQ],
                            start=(kbb == 0),
                            stop=(kbb == nkb - 1),
                        )

            def attention_pair(mt, qc, fillers, slots_left, burst):
                nkb = 4 * qc + 4
                av = [ps_av.tile([P, NQ], F32, tag="av", name=f"av{mt}_{qc}_{i}")
                      for i in range(2)]
                pts = []
                for kb in range(nkb):
                    t = P * (kb - 4 * qc)  # <0 for full blocks
                    lo = max(t, 0)
                    s_ps = ps_s.tile([P, 2 * NQ], F32, tag="s",
                                     name=f"s{mt}_{qc}_{kb}")
                    s3 = s_ps.rearrange("p (h n) -> p h n", n=NQ)
                    for h2 in range(2):
                        nc.tensor.matmul(
                            s3[:, h2, lo:NQ],
                            lhsT=kTt[mt][kb // 4][h2 * D_K:(h2 + 1) * D_K,
                                                 (kb % 4) * P:(kb % 4 + 1) * P],
                            rhs=qTt[mt][qc][h2 * D_K:(h2 + 1) * D_K, lo:NQ],
                            start=True,
                            stop=True,
                        )
                    pt = pt_pool.tile([P, 2 * NQ], BF16, tag="pt",
                                      name=f"pt{mt}_{qc}_{kb}")
                    p3 = pt.rearrange("p (h n) -> p h n", n=NQ)
                    if t <= 0:
                        nc.scalar.activation(out=pt, in_=s_ps, func=AF.Exp)
                    else:
                        nc.scalar.activation(out=p3[:, :, t:NQ],
                                             in_=s3[:, :, t:NQ], func=AF.Exp)
                    if t >= 0:  # diagonal sub-block: triangular mask
                        for h2 in range(2):
                            nc.vector.tensor_tensor(
                                out=p3[:, h2, t:t + P],
                                in0=p3[:, h2, t:t + P],
                                in1=mask_sb,
                                op=ALU.mult,
                            )
                    pts.append((p3, lo, kb))
                    # interleave independent projection / o_proj matmuls so
                    # the tensor queue has work while exp(kb) runs
                    npop = 0 if slots_left[0] <= 0 else (
                        (len(fillers) + slots_left[0] - 1) // slots_left[0])
                    slots_left[0] -= 1
                    for _ in range(npop):
                        if fillers:
                            fillers.pop(0)()
                    # lookahead-1: AV for the previous block
                    if kb > 0:
                        av_batch(av, mt, pts[kb - 1:kb], nkb)
                av_batch(av, mt, pts[nkb - 1:nkb], nkb)
                # normalize: rows 0:64 of av hold the replicated denominator
                for h2 in range(2):
                    rec = rec_pool.tile([D_K, NQ], F32, tag="rec",
                                        name=f"rec{mt}_{qc}_{h2}")
                    nc.vector.reciprocal_approx_fast(rec, av[h2][0:D_K, :])
                    nc.vector.tensor_tensor(
                        out=aoTq[mt][qc][h2 * D_K:(h2 + 1) * D_K, :],
                        in0=av[h2][D_K:2 * D_K, :],
                        in1=rec,
                        op=ALU.mult,
                    )

            # ---- emission schedule ----
            # prologue: qc=0 projections (k, v tiles 0..3, q)
            for mt in range(N_MT):
                for op in kq_chain(wk_sb, bk_sb, kTt, xkc, "k", 0, mt):
                    op()
            for tt in range(4):
                for op in v_chain(tt):
                    op()
            for mt in range(N_MT):
                for op in kq_chain(wq_sb, bq_sb, qTt, xqc, "q", 0, mt):
                    op()

            for qc in range(N_QC):
                fillers = []
                if qc < 3:
                    nz = qc + 1
                    # next-wave input chunks (issue queues: k/q on gpsimd,
                    # v on vector, weights done)
                    x_chunk_dmas(xkc, xkT, "k", nz, nc.sync)
                    x_chunk_dmas(xvc, xvT, "v", nz, nc.sync)
                    x_chunk_dmas(xqc, xqT, "q", nz, nc.sync)
                    for mt in range(N_MT):
                        fillers += kq_chain(wk_sb, bk_sb, kTt, xkc, "k", nz, mt)
                        fillers += v_chain(4 * nz + mt)
                        fillers += kq_chain(wq_sb, bq_sb, qTt, xqc, "q", nz, mt)
                else:
                    for oqc in range(3):
                        for j in range(4):
                            for dc in range(2):
                                fillers += o_chain(oqc, j, dc)
                slots_left = [N_MT * (4 * qc + 4)]
                burst = [0]
                for mt in range(N_MT):
                    attention_pair(mt, qc, fillers, slots_left, burst)
                for op in fillers:  # drain any stragglers
                    op()
            for j in range(4):
                for dc in range(2):
                    for op in o_chain(3, j, dc):
                        op()
    nc.finalize()
    return nc


def make_in_maps(Q, K, V, Wq, bq, Wk, bk, Wv, bv, Wo, bo, attn_mask=None):
    """Build the 8 per-core input maps from full (unsharded) inputs."""
    Q = np.asarray(Q, np.float32)
    K = np.asarray(K, np.float32)
    V = np.asarray(V, np.float32)
    scale = np.float32(1.0 / np.sqrt(np.float32(D_K)))
    Wq = np.asarray(Wq, np.float32) * scale   # fold 1/sqrt(d_k) into Wq/bq
    bq = np.asarray(bq, np.float32) * scale
    Wk = np.asarray(Wk, np.float32)
    Wv = np.asarray(Wv, np.float32)
    Wo = np.asarray(Wo, np.float32)
    bk = np.asarray(bk, np.float32)
    bv = np.asarray(bv, np.float32)

    i_idx = np.arange(P)[:, None]
    j_idx = np.arange(P)[None, :]
    maskc = (i_idx <= j_idx).astype(NPBF16)

    xT = {}
    for b in range(B):
        xT[b] = tuple(
            np.ascontiguousarray(X[b].T).astype(NPBF16) for X in (Q, K, V)
        )
    grp = {}
    for g in range(TP):
        sl = slice(g * GD, (g + 1) * GD)
        grp[g] = dict(
            wqT=np.ascontiguousarray(Wq[sl, :].T).astype(NPBF16),
            wkT=np.ascontiguousarray(Wk[sl, :].T).astype(NPBF16),
            wvT=np.ascontiguousarray(Wv[sl, :].T).astype(NPBF16),
            woT=np.ascontiguousarray(Wo[:, sl].T).astype(NPBF16),
            bq=np.ascontiguousarray(bq[sl].reshape(N_MT, P).T).astype(np.float32),
            bk=np.ascontiguousarray(bk[sl].reshape(N_MT, P).T).astype(np.float32),
            bv=np.ascontiguousarray(bv[sl].reshape(1, GD)).astype(np.float32),
        )
    in_maps = []
    for c in range(2 * B):
        b, g = c // 2, c % 2
        m = dict(grp[g])
        m["xqT"], m["xkT"], m["xvT"] = xT[b]
        m["maskc"] = maskc
        in_maps.append(m)
    return in_maps


def assemble_output(results, bo):
    bo = np.asarray(bo, np.float32)
    out = np.empty((B, L, D_MODEL), np.float32)
    for b in range(B):
        out[b] = results[2 * b]["out"] + results[2 * b + 1]["out"] + bo
    return out


_NC_CACHE = None


def kernel(**inputs) -> np.ndarray:
    global _NC_CACHE
    from concourse.bass_utils import run_bass_kernel_spmd

    if _NC_CACHE is None:
        _NC_CACHE = build_nc()
    in_maps = make_in_maps(**inputs)
    res = run_bass_kernel_spmd(_NC_CACHE, in_maps, core_ids=list(range(2 * B)))
    return assemble_output(res.results, inputs["bo"])


# revision 38
# speedup vs baseline: 1.0193x; 1.0193x over previous
"""Multi-head causal attention (QKV proj + attention + O proj) on 8 TRN2 cores.

Sharding: data-parallel over batch (4) x tensor-parallel over heads (2 groups
of 8 heads).  Core c handles batch c//2, head-group c%2.  Each core computes
its group's partial o_proj output; the host sums the two partials per batch.

Layout strategy (all activations arrive pre-transposed from the host):
  - qT, kT per head-pair M-tile: (128 head-dims, L) from  W.T-slice @ X.T
    (1/sqrt(d_k) is folded into Wq/bq on the host).
  - vA natural (tokens, head-dims) with 64 REPLICATED ones-columns per head:
    av_psum = vA.T @ P gives (128, Nq) where rows 64:128 all hold the per-query
    sum of probabilities.  Normalization = vector reciprocal of rows 64:128 +
    one elementwise mult -- no DRAM round trips, no cross-partition broadcast.
  - AV accumulates over ALL key blocks in a single PSUM bank (no SBUF
    eviction), exp without max-subtraction (scores are O(+-6), safe in fp32),
    causal masking multiplies a small triangular mask after exp.
  - Attention inner loop is software-pipelined: scores(kb+1) and interleaved
    projection matmuls are emitted between scores(kb) and AV(kb) so the
    in-order tensor queue never stalls behind the scalar-engine exp.
Compute dtype bf16 (fp32 PSUM accumulation); fp32 partial outputs.
"""

import numpy as np
import ml_dtypes

import concourse.bass as bass
import concourse.tile as tile
from concourse import bacc, mybir

D_MODEL = 1024
N_HEADS = 16
D_K = 64
B, L = 4, 2048
TP = 2                  # head groups
GD = D_MODEL // TP      # 512 head-dims per group
P = 128
NQ = 512                # query chunk (one fp32 PSUM bank)
N_MT = GD // P          # 4 M-tiles (head pairs) per group
N_KT = D_MODEL // P     # 8 contraction tiles over model dim
N_TT = L // P           # 16 token tiles
N_QC = L // NQ          # 4 query chunks
BF16 = mybir.dt.bfloat16
F32 = mybir.dt.float32
NPBF16 = ml_dtypes.bfloat16
AF = mybir.ActivationFunctionType
ALU = mybir.AluOpType


def build_nc() -> bass.Bass:
    nc = bacc.Bacc("TRN2", target_bir_lowering=False)

    xqT = nc.dram_tensor("xqT", [D_MODEL, L], BF16, kind="ExternalInput")
    xkT = nc.dram_tensor("xkT", [D_MODEL, L], BF16, kind="ExternalInput")
    xvT = nc.dram_tensor("xvT", [D_MODEL, L], BF16, kind="ExternalInput")
    wqT = nc.dram_tensor("wqT", [D_MODEL, GD], BF16, kind="ExternalInput")
    wkT = nc.dram_tensor("wkT", [D_MODEL, GD], BF16, kind="ExternalInput")
    wvT = nc.dram_tensor("wvT", [D_MODEL, GD], BF16, kind="ExternalInput")
    woT = nc.dram_tensor("woT", [GD, D_MODEL], BF16, kind="ExternalInput")
    bq = nc.dram_tensor("bq", [P, N_MT], F32, kind="ExternalInput")
    bk = nc.dram_tensor("bk", [P, N_MT], F32, kind="ExternalInput")
    bv = nc.dram_tensor("bv", [1, GD], F32, kind="ExternalInput")
    maskc = nc.dram_tensor("maskc", [P, P], BF16, kind="ExternalInput")
    out = nc.dram_tensor("out", [L, D_MODEL], F32, kind="ExternalOutput")

    with tile.TileContext(nc) as tc:
        with (
            tc.tile_pool(name="const", bufs=1) as const,
            tc.tile_pool(name="xch", bufs=32) as xch_pool,
            tc.tile_pool(name="xvc", bufs=16) as xvc_pool,
            tc.tile_pool(name="pt", bufs=6) as pt_pool,
            tc.tile_pool(name="rec", bufs=4) as rec_pool,
            tc.tile_pool(name="osb", bufs=4) as osb_pool,
            tc.tile_pool(name="ps_s", bufs=2, space="PSUM") as ps_s,
            tc.tile_pool(name="ps_av", bufs=2, space="PSUM") as ps_av,
            tc.tile_pool(name="ps_mm", bufs=2, space="PSUM") as ps_mm,
        ):
            # ---- resident weights / constants ----
            wk_sb = const.tile([P, N_KT, GD], BF16, tag="wk")
            wv_sb = const.tile([P, N_KT, GD], BF16, tag="wv")
            wq_sb = const.tile([P, N_KT, GD], BF16, tag="wq")
            wo_sb = const.tile([P, N_MT, D_MODEL], BF16, tag="wo")
            bk_sb = const.tile([P, N_MT], F32, tag="bk")
            bq_sb = const.tile([P, N_MT], F32, tag="bq")
            bv_sb = const.tile([P, GD], F32, tag="bv")
            mask_sb = const.tile([P, P], BF16, tag="mask")

            # per-(mt, chunk) activation tiles
            qTt = [[const.tile([P, NQ], BF16, tag=f"qT{mt}_{ncz}", name=f"qT{mt}_{ncz}")
                    for ncz in range(N_QC)] for mt in range(N_MT)]
            kTt = [[const.tile([P, NQ], BF16, tag=f"kT{mt}_{ncz}", name=f"kT{mt}_{ncz}")
                    for ncz in range(N_QC)] for mt in range(N_MT)]
            # v with 64 replicated ones-columns per head: [tok, tt, head, 128]
            vA = const.tile([P, N_TT, 2 * N_MT, 2 * D_K], BF16, tag="vA")
            aoTq = [[const.tile([P, NQ], BF16, tag=f"ao{mt}_{qc}", name=f"ao{mt}_{qc}")
                     for qc in range(N_QC)] for mt in range(N_MT)]

            # ---- DMA issue: interleave weight/activation chunks so the first
            # projection chains start after ~256KB, not ~2MB; keep the scalar
            # queue free for exp ----
            nc.sync.dma_start(out=mask_sb, in_=maskc[:, :])
            nc.sync.dma_start(out=bk_sb, in_=bk[:, :])
            nc.sync.dma_start(out=bq_sb, in_=bq[:, :])
            nc.sync.dma_start(out=bv_sb, in_=bv[:, :].to_broadcast([P, GD]))

            xkc = {}   # (qc, kt) -> tile
            xqc = {}
            xvc = {}

            def x_chunk_dmas(dst, x_dram, nm, qc, eng):
                for kt in range(N_KT):
                    xc = (xvc_pool if nm == "v" else xch_pool).tile(
                        [P, NQ], BF16, tag="xvc" if nm == "v" else "xch",
                        name=f"x{nm}{qc}_{kt}")
                    eng.dma_start(
                        out=xc,
                        in_=x_dram[kt * P:(kt + 1) * P, qc * NQ:(qc + 1) * NQ],
                    )
                    dst[(qc, kt)] = xc

            # wk on gpsimd concurrently with xk0 on sync; then v, q inputs
            x_chunk_dmas(xkc, xkT, "k", 0, nc.sync)
            for kt in range(N_KT):
                nc.gpsimd.dma_start(out=wk_sb[:, kt, :],
                                    in_=wkT[kt * P:(kt + 1) * P, :])
            for kt in range(N_KT):
                nc.sync.dma_start(out=wv_sb[:, kt, :],
                                  in_=wvT[kt * P:(kt + 1) * P, :])
            x_chunk_dmas(xvc, xvT, "v", 0, nc.sync)
            x_chunk_dmas(xqc, xqT, "q", 0, nc.gpsimd)
            for kt in range(N_KT):
                nc.sync.dma_start(out=wq_sb[:, kt, :],
                                  in_=wqT[kt * P:(kt + 1) * P, :])
            for kt in range(N_MT):
                nc.sync.dma_start(out=wo_sb[:, kt, :],
                                  in_=woT[kt * P:(kt + 1) * P, :])

            # ones block of vA (cols 0:64 -> av rows 0:64 = replicated denom)
            nc.gpsimd.memset(vA[:, 0:N_TT // 4, :, 0:D_K], 1.0)
            nc.vector.memset(vA[:, N_TT // 4:N_TT, :, 0:D_K], 1.0)

            # ---- projection helpers ----
            def kq_chain(w_sb, b_sb, dsts, src, nm, qc, mt):
                """Returns list of emission closures: 8 matmuls + 1 evict."""
                ops = []
                ps = ps_mm.tile([P, NQ], F32, tag="mm", name=f"ps{nm}{qc}{mt}")

                def mm(kt):
                    def go():
                        nc.tensor.matmul(
                            ps,
                            lhsT=w_sb[:, kt, mt * P:(mt + 1) * P],
                            rhs=src[(qc, kt)],
                            start=(kt == 0),
                            stop=(kt == N_KT - 1),
                        )
                    return go
                for kt in range(N_KT):
                    ops.append(mm(kt))

                def evict():
                    nc.vector.tensor_scalar_add(
                        dsts[mt][qc], ps, b_sb[:, mt:mt + 1])
                ops.append(evict)
                return ops

            def v_chain(tt):
                """8 matmuls + 1 biased evict into vA[:, tt, :, 0:D_K]."""
                ops = []
                qc, j = tt // 4, tt % 4
                ps = ps_mm.tile([P, GD], F32, tag="mm", name=f"psv{tt}")

                def mm(kt):
                    def go():
                        nc.tensor.matmul(
                            ps,
                            lhsT=xvc[(qc, kt)][:, j * P:(j + 1) * P],
                            rhs=wv_sb[:, kt, :],
                            start=(kt == 0),
                            stop=(kt == N_KT - 1),
                        )
                    return go
                for kt in range(N_KT):
                    ops.append(mm(kt))

                def evict():
                    nc.vector.tensor_tensor(
                        out=vA[:, tt, :, D_K:2 * D_K],
                        in0=ps.rearrange("p (h d) -> p h d", d=D_K),
                        in1=bv_sb.rearrange("p (h d) -> p h d", d=D_K),
                        op=ALU.add,
                    )
                ops.append(evict)
                return ops

            def o_chain(qc, j, dc):
                """4 matmuls + 1 evict+store for out rows (4qc+j)*P, cols dc*NQ."""
                ops = []
                lt = 4 * qc + j
                ps = ps_mm.tile([P, NQ], F32, tag="mm", name=f"po{lt}_{dc}")

                def mm(kt):
                    def go():
                        nc.tensor.matmul(
                            ps,
                            lhsT=aoTq[kt][qc][:, j * P:(j + 1) * P],
                            rhs=wo_sb[:, kt, dc * NQ:(dc + 1) * NQ],
                            start=(kt == 0),
                            stop=(kt == N_MT - 1),
                        )
                    return go
                for kt in range(N_MT):
                    ops.append(mm(kt))

                def evict():
                    ot = osb_pool.tile([P, NQ], F32, tag="ot", name=f"ot{lt}_{dc}")
                    nc.vector.tensor_copy(out=ot, in_=ps)
                    nc.sync.dma_start(
                        out=out[lt * P:(lt + 1) * P, dc * NQ:(dc + 1) * NQ],
                        in_=ot,
                    )
                ops.append(evict)
                return ops

            # ---- attention with software-pipelined emission; `fillers` is a
            # shared list of closures drained evenly across the wave ----
            def av_batch(av, mt, batch, nkb):
                for h2 in range(2):
                    for (p3b, lob, kbb) in batch:
                        nc.tensor.matmul(
                            av[h2][:, lob:NQ],
                            lhsT=vA[:, kbb, 2 * mt + h2, :],
                            rhs=p3b[:, h2, lob:NQ],
                            start=(kbb == 0),
                            stop=(kbb == nkb - 1),
                        )

            def attention_pair(mt, qc, fillers, slots_left, burst):
                nkb = 4 * qc + 4
                av = [ps_av.tile([P, NQ], F32, tag="av", name=f"av{mt}_{qc}_{i}")
                      for i in range(2)]
                pts = []
                for kb in range(nkb):
                    t = P * (kb - 4 * qc)  # <0 for full blocks
                    lo = max(t, 0)
                    s_ps = ps_s.tile([P, 2 * NQ], F32, tag="s",
                                     name=f"s{mt}_{qc}_{kb}")
                    s3 = s_ps.rearrange("p (h n) -> p h n", n=NQ)
                    for h2 in range(2):
                        nc.tensor.matmul(
                            s3[:, h2, lo:NQ],
                            lhsT=kTt[mt][kb // 4][h2 * D_K:(h2 + 1) * D_K,
                                                 (kb % 4) * P:(kb % 4 + 1) * P],
                            rhs=qTt[mt][qc][h2 * D_K:(h2 + 1) * D_K, lo:NQ],
                            start=True,
                            stop=True,
                        )
                    pt = pt_pool.tile([P, 2 * NQ], BF16, tag="pt",
                                      name=f"pt{mt}_{qc}_{kb}")
                    p3 = pt.rearrange("p (h n) -> p h n", n=NQ)
                    if t <= 0:
                        nc.scalar.activation(out=pt, in_=s_ps, func=AF.Exp)
                    else:
                        nc.scalar.activation(out=p3[:, :, t:NQ],
                                             in_=s3[:, :, t:NQ], func=AF.Exp)
                    if t >= 0:  # diagonal sub-block: triangular mask
                        for h2 in range(2):
                            nc.vector.tensor_tensor(
                                out=p3[:, h2, t:t + P],
                                in0=p3[:, h2, t:t + P],
                                in1=mask_sb,
                                op=ALU.mult,
                            )
                    pts.append((p3, lo, kb))
                    # interleave independent projection / o_proj matmuls so
                    # the tensor queue has work while exp(kb) runs
                    npop = 0 if slots_left[0] <= 0 else (
                        (len(fillers) + slots_left[0] - 1) // slots_left[0])
                    slots_left[0] -= 1
                    for _ in range(npop):
                        if fillers:
                            fillers.pop(0)()
                    # lookahead-1: AV for the previous block
                    if kb > 0:
                        av_batch(av, mt, pts[kb - 1:kb], nkb)
                av_batch(av, mt, pts[nkb - 1:nkb], nkb)
                # normalize: rows 0:64 of av hold the replicated denominator
                for h2 in range(2):
                    rec = rec_pool.tile([D_K, NQ], F32, tag="rec",
                                        name=f"rec{mt}_{qc}_{h2}")
                    nc.vector.reciprocal_approx_fast(rec, av[h2][0:D_K, :])
                    nc.vector.tensor_tensor(
                        out=aoTq[mt][qc][h2 * D_K:(h2 + 1) * D_K, :],
                        in0=av[h2][D_K:2 * D_K, :],
                        in1=rec,
                        op=ALU.mult,
                    )

            # ---- emission schedule ----
            # prologue: qc=0 projections (k, v tiles 0..3, q)
            for mt in range(N_MT):
                for op in kq_chain(wk_sb, bk_sb, kTt, xkc, "k", 0, mt):
                    op()
            for tt in range(4):
                for op in v_chain(tt):
                    op()
            for mt in range(N_MT):
                for op in kq_chain(wq_sb, bq_sb, qTt, xqc, "q", 0, mt):
                    op()

            for qc in range(N_QC):
                fillers = []
                if qc < 3:
                    nz = qc + 1
                    # next-wave input chunks (issue queues: k/q on gpsimd,
                    # v on vector, weights done)
                    x_chunk_dmas(xkc, xkT, "k", nz, nc.sync)
                    x_chunk_dmas(xvc, xvT, "v", nz, nc.sync)
                    x_chunk_dmas(xqc, xqT, "q", nz, nc.sync)
                    for mt in range(N_MT):
                        fillers += kq_chain(wk_sb, bk_sb, kTt, xkc, "k", nz, mt)
                        fillers += v_chain(4 * nz + mt)
                        fillers += kq_chain(wq_sb, bq_sb, qTt, xqc, "q", nz, mt)
                else:
                    for oqc in range(3):
                        for j in range(4):
                            for dc in range(2):
                                fillers += o_chain(oqc, j, dc)
                slots_left = [N_MT * (4 * qc + 4)]
                burst = [0]
                for mt in range(N_MT):
                    attention_pair(mt, qc, fillers, slots_left, burst)
                for op in fillers:  # drain any stragglers
                    op()
            for j in range(4):
                for dc in range(2):
                    for op in o_chain(3, j, dc):
                        op()
    nc.finalize()
    return nc


def make_in_maps(Q, K, V, Wq, bq, Wk, bk, Wv, bv, Wo, bo, attn_mask=None):
    """Build the 8 per-core input maps from full (unsharded) inputs."""
    Q = np.asarray(Q, np.float32)
    K = np.asarray(K, np.float32)
    V = np.asarray(V, np.float32)
    scale = np.float32(1.0 / np.sqrt(np.float32(D_K)))
    Wq = np.asarray(Wq, np.float32) * scale   # fold 1/sqrt(d_k) into Wq/bq
    bq = np.asarray(bq, np.float32) * scale
    Wk = np.asarray(Wk, np.float32)
    Wv = np.asarray(Wv, np.float32)
    Wo = np.asarray(Wo, np.float32)
    bk = np.asarray(bk, np.float32)
    bv = np.asarray(bv, np.float32)

    i_idx = np.arange(P)[:, None]
    j_idx = np.arange(P)[None, :]
    maskc = (i_idx <= j_idx).astype(NPBF16)

    xT = {}
    for b in range(B):
        xT[b] = tuple(
            np.ascontiguousarray(X[b].T).astype(NPBF16) for X in (Q, K, V)
        )
    grp = {}
    for g in range(TP):
        sl = slice(g * GD, (g + 1) * GD)
        grp[g] = dict(
            wqT=np.ascontiguousarray(Wq[sl, :].T).astype(NPBF16),
            wkT=np.ascontiguousarray(Wk[sl, :].T).astype(NPBF16),
            wvT=np.ascontiguousarray(Wv[sl, :].T).astype(NPBF16),
            woT=np.ascontiguousarray(Wo[:, sl].T).astype(NPBF16),
            bq=np.ascontiguousarray(bq[sl].reshape(N_MT, P).T).astype(np.float32),
            bk=np.ascontiguousarray(bk[sl].reshape(N_MT, P).T).astype(np.float32),
            bv=np.ascontiguousarray(bv[sl].reshape(1, GD)).astype(np.float32),
        )
    in_maps = []
    for c in range(2 * B):
        b, g = c // 2, c % 2
        m = dict(grp[g])
        m["xqT"], m["xkT"], m["xvT"] = xT[b]
        m["maskc"] = maskc
        in_maps.append(m)
    return in_maps


def assemble_output(results, bo):
    bo = np.asarray(bo, np.float32)
    out = np.empty((B, L, D_MODEL), np.float32)
    for b in range(B):
        out[b] = results[2 * b]["out"] + results[2 * b + 1]["out"] + bo
    return out


_NC_CACHE = None


def kernel(**inputs) -> np.ndarray:
    global _NC_CACHE
    from concourse.bass_utils import run_bass_kernel_spmd

    if _NC_CACHE is None:
        _NC_CACHE = build_nc()
    in_maps = make_in_maps(**inputs)
    res = run_bass_kernel_spmd(_NC_CACHE, in_maps, core_ids=list(range(2 * B)))
    return assemble_output(res.results, inputs["bo"])
